# revision 10
# baseline (speedup 1.0000x reference)
"""dX-privacy embedding snap (argmax over vocab of noisy-embedding scores)
for Trainium2, 8 NeuronCores.

Distribution: vocab-sharded. Core c owns rows [c*4000, (c+1)*4000) and
scores all 8192 tokens against its shard.

Device math runs in fp8e4 (e4m3) with perf_mode=DoubleRow: each matmul
contracts 256 dims (2 fp8 rows per PE cell) over 500 vocab columns (stored
at stride 512 to satisfy DoubleRow's step%16 AP rule), all 8 PSUM banks
accumulate one 128-token tile x 4000 vocab scores over 16 k-pair steps. The
whole E^T shard (~16.8 MB fp8) stays resident in SBUF; x^T tiles stream per
128-token block. DVE max/max_index emit top-8 (value, index) per token per
core.

fp8 rounding gives score error sigma ~= 2.8 on D=4096 dot products vs a mean
top-1/top-2 gap of ~14.5, so the DEVICE top-8 is only a candidate filter:
the true winner survives into the 64 host-merged candidates unless >=8 rows
of its own shard out-noise it (P ~ 1e-10). The host then rescores the top-16
candidates per token exactly in fp32 (chunked gather + einsum) and keeps the
jnp full-vocab fp32 rescore (reference arithmetic) for tokens whose exact
margin < 0.05, which makes the final argmax match the fp32 reference.
"""

import sys, os, time

sys.path.insert(0, "/opt/trn_rl_repo")
import numpy as np

import bass_rust
import concourse.bass as bass
import concourse.mybir as mybir
from concourse import tile


f32 = mybir.dt.float32
f16 = mybir.dt.float16
f8 = mybir.dt.float8e4
u32 = mybir.dt.uint32

B, S, D, V = 4, 2048, 4096, 32000
T = B * S  # 8192 tokens
N_CORES = 8
VSH = V // N_CORES  # 4000 vocab rows per core (no padding)
KP = D // 256  # 16 k-pair steps (DoubleRow contracts 256 dims/matmul)
NVB = 8  # v blocks per core (one PSUM bank each)
NV = 500  # live v block width (stored at stride 512: DoubleRow step%16==0)
NVS = 512  # storage stride of a v block half
NTG = T // 128  # 64 token tiles of 128
THETA = 0.05  # host full-rescore margin on exact fp32 candidate scores
TOPC = 16  # candidates exactly rescored per token

_mwfix_ctr = [0]


def _legalize_multiwaits(nc, max_waits=1):
    """walrus encodes at most one sem wait per instruction; split multi-wait
    instructions by inserting single-wait NOPs before them (same engine)."""
    for fn in nc.m.functions:
        for bb in fn.blocks:
            insts = list(bb.instructions)
            out = []
            changed = False
            for inst in insts:
                si = inst.sync_info
                ow = list(si.on_wait) if si is not None and si.on_wait else []
                if len(ow) > max_waits:
                    for wentry in ow[:-max_waits]:
                        _mwfix_ctr[0] += 1
                        nop = mybir.InstNoOp(
                            name=f"mwfix-{_mwfix_ctr[0]}", ins=[], outs=[]
                        )
                        nop.engine = inst.engine
                        nop.sync_info = bass_rust.SyncInfo(
                            on_wait=[wentry], on_update=[]
                        )
                        out.append(nop)
                    si.on_wait = ow[-max_waits:]
                    changed = True
                out.append(inst)
            if changed:
                bb.instructions = out


def _build_nc():
    nc = bass.Bass()
    # xt[tg, p, kp*2*128]: xt[tg, p, kp, i, t] = x8[tg*128+t, kp*256+i*128+p]
    xt_in = nc.declare_dram_parameter("xt", [NTG, 128, KP, 2, 128], f8, isOutput=False)
    # et[p, kp, vb, i, v<500] = E8[vb*500+v, kp*256+i*128+p] for this core's
    # shard; v is stored at stride 512 (cols 500..511 are zero padding)
    et_in = nc.declare_dram_parameter("et", [128, KP, NVB, 2, NVS], f8, isOutput=False)
    out_val = nc.declare_dram_parameter("val8", [128, NTG * 8], f32, isOutput=True)
    out_idx = nc.declare_dram_parameter("idx8", [128, NTG * 8], u32, isOutput=True)

    DR = mybir.MatmulPerfMode.DoubleRow

    with tile.TileContext(nc) as tc:
        with (
            tc.tile_pool(name="et", bufs=1) as etp,
            tc.tile_pool(name="xt", bufs=3) as xtp,
            tc.tile_pool(name="bnc", bufs=2) as bncp,
            tc.tile_pool(name="o8", bufs=1) as o8p,
            tc.tile_pool(name="ps", bufs=1, space="PSUM") as ps,
        ):
            # resident E^T shard: 128 KiB per partition
            et_sb = etp.tile([128, KP, NVB, 2, NVS], f8, name="et_sb")
            # kp-granular loads so tg=0 compute starts after the first chunk
            for kp in range(KP):
                nc.sync.dma_start(et_sb[:, kp], et_in[:, kp])

            val_acc = o8p.tile([128, NTG, 8], f32, name="val_acc")
            idx_acc = o8p.tile([128, NTG, 8], u32, name="idx_acc")

            for tg in range(NTG):
                xt_t = xtp.tile([128, KP, 2, 128], f8, tag="xt", name=f"xt_{tg}")
                nc.scalar.dma_start(xt_t[:], xt_in[tg])

                psums = []
                for vb in range(NVB):
                    pst = ps.tile([128, NV], f32, tag=f"ps{vb}", name=f"ps_{tg}_{vb}")
                    psums.append(pst)
                for kp in range(KP):
                    for vb in range(NVB):
                        mm = nc.tensor.matmul(
                            psums[vb][:],
                            xt_t[:, kp],
                            et_sb[:, kp, vb, :, :NV],
                            start=(kp == 0),
                            stop=(kp == KP - 1),
                            perf_mode=DR,
                        )
                        if vb > 0:
                            # same stationary as vb=0: skip the reload
                            mm.ldweights = False
                # bounce scores out of PSUM so the banks recycle for tg+1;
                # top-8 over the full 4096-wide row off the SBUF copy
                bsc = bncp.tile([128, NVB * NV], f32, tag="bn", name=f"bn_{tg}")
                for vb in range(NVB):
                    nc.vector.tensor_copy(
                        bsc[:, vb * NV : (vb + 1) * NV], psums[vb][:]
                    )
                nc.vector.max(out=val_acc[:, tg], in_=bsc[:])
                nc.vector.max_index(
                    out=idx_acc[:, tg], in_max=val_acc[:, tg], in_values=bsc[:]
                )
            # ACT queue: keep result writes out of the E^T load FIFO
            nc.scalar.dma_start(out_val[:], val_acc[:])
            nc.scalar.dma_start(out_idx[:], idx_acc[:])
    _legalize_multiwaits(nc)
    return nc


_RUNNER = None
LAST_TIMES = None  # per-call wall times of the timed iterations


def _get_runner():
    global _RUNNER
    if _RUNNER is not None:
        return _RUNNER
    import jax
    from jax.sharding import Mesh, PartitionSpec, NamedSharding
    from jax.experimental.shard_map import shard_map
    from concourse.bass2jax import (
        _bass_exec_p,
        install_neuronx_cc_hook,
        partition_id_tensor,
    )

    nc = _build_nc()
    install_neuronx_cc_hook()
    partition_name = nc.partition_id_tensor.name if nc.partition_id_tensor else None

    in_names, out_names, out_avals, zero_outs = [], [], [], []
    for alloc in nc.m.functions[0].allocations:
        if not isinstance(alloc, mybir.MemoryLocationSet):
            continue
        name = alloc.memorylocations[0].name
        if alloc.kind == "ExternalInput":
            if name != partition_name:
                in_names.append(name)
        elif alloc.kind == "ExternalOutput":
            shape, dt = alloc.tensor_shape, mybir.dt.np(alloc.dtype)
            out_names.append(name)
            out_avals.append(jax.core.ShapedArray(shape, dt))
            zero_outs.append(np.zeros(shape, dt))

    n_params = len(in_names)
    all_in_names = list(in_names) + list(out_names)
    if partition_name is not None:
        all_in_names.append(partition_name)

    def _body(*args):
        operands = list(args)
        if partition_name is not None:
            operands.append(partition_id_tensor())
        outs = _bass_exec_p.bind(
            *operands,
            out_avals=tuple(out_avals),
            in_names=tuple(all_in_names),
            out_names=tuple(out_names),
            lowering_input_output_aliases=(),
            sim_require_finite=True,
            sim_require_nnan=True,
            nc=nc,
        )
        return tuple(outs)

    devices = jax.devices()[:N_CORES]
    mesh = Mesh(np.asarray(devices), ("core",))
    in_specs = (PartitionSpec("core"),) * (n_params + len(out_names))
    out_specs = (PartitionSpec("core"),) * len(out_names)
    fn = jax.jit(
        shard_map(
            _body, mesh=mesh, in_specs=in_specs, out_specs=out_specs, check_rep=False
        ),
        keep_unused=True,
    )

    def run(in_maps, n_iters=1):
        global LAST_TIMES
        args = []
        for name in in_names:
            shards = [
                jax.device_put(np.ascontiguousarray(in_maps[c][name]), devices[c])
                for c in range(N_CORES)
            ]
            per_shape = shards[0].shape
            gshape = (N_CORES * per_shape[0],) + tuple(per_shape[1:])
            args.append(
                jax.make_array_from_single_device_arrays(
                    gshape, NamedSharding(mesh, PartitionSpec("core")), shards
                )
            )
        zargs = []
        for z in zero_outs:
            shards = [jax.device_put(z, d) for d in devices]
            gshape = (N_CORES * z.shape[0],) + tuple(z.shape[1:])
            zargs.append(
                jax.make_array_from_single_device_arrays(
                    gshape, NamedSharding(mesh, PartitionSpec("core")), shards
                )
            )
        out = fn(*args, *zargs)
        jax.block_until_ready(out)
        globals()["_FN"] = fn
        globals()["_ARGS"] = (args, zargs)
        times = []
        for _ in range(n_iters - 1):
            t0 = time.perf_counter()
            out = fn(*args, *zargs)
            jax.block_until_ready(out)
            times.append(time.perf_counter() - t0)
        LAST_TIMES = times
        results = []
        for c in range(N_CORES):
            m = {}
            for i, name in enumerate(out_names):
                ga = np.asarray(out[i]).reshape((N_CORES,) + out_avals[i].shape)
                m[name] = ga[c]
            results.append(m)
        return results

    _RUNNER = run
    return run


def measure_exec_ns(chains=(20, 120), tries=3):
    """Per-execution device time via long-chain slope (cancels the tunnel's
    ~40ms completion-poll quantization). Requires a prior kernel() call."""
    import jax

    fn = globals().get("_FN")
    args, zargs = globals().get("_ARGS")
    best = None
    for _ in range(tries):
        ts = []
        for n in chains:
            o = fn(*args, *zargs)
            jax.block_until_ready(o)  # sync point
            t0 = time.perf_counter()
            for _ in range(n):
                o = fn(*args, *zargs)
            jax.block_until_ready(o)
            ts.append(time.perf_counter() - t0)
        per = (ts[1] - ts[0]) / (chains[1] - chains[0])
        best = per if best is None else min(best, per)
    return best * 1e9


def kernel(inputs_embeds, embed_table, noise):
    verbose = os.environ.get("KERNEL_VERBOSE")
    _t = [time.time()]

    def _lap(msg):
        if verbose:
            t = time.time()
            print(f"[kernel] {msg}: {t - _t[0]:.1f}s", flush=True)
            _t[0] = t

    inputs_embeds = np.asarray(inputs_embeds)
    embed_table = np.asarray(embed_table)
    noise = np.asarray(noise)
    f8np = mybir.dt.np(f8)

    # host prep (layout only)
    x = (inputs_embeds + noise).reshape(T, D).astype(np.float32)
    x8 = x.astype(f8np)
    # xt[tg, p, kp, i, t] = x8[tg*128+t, kp*256+i*128+p]
    xt = np.ascontiguousarray(
        x8.reshape(NTG, 128, KP, 2, 128).transpose(0, 4, 2, 3, 1)
    )
    e8 = embed_table.astype(f8np)
    in_maps = []
    for c in range(N_CORES):
        sh = e8[c * VSH : (c + 1) * VSH]  # [4000 rows, 4096 dims] fp8
        # et[p, kp, vb, i, v<500] = sh[vb*500+v, kp*256+i*128+p]
        et = np.zeros((128, KP, NVB, 2, NVS), dtype=f8np)
        et[..., :NV] = sh.reshape(NVB, NV, KP, 2, 128).transpose(4, 2, 0, 3, 1)
        in_maps.append({"xt": xt, "et": et})
    _lap("host prep")

    run = _get_runner()
    _lap("compile/runner")
    n_iters = int(os.environ.get("KERNEL_TIME_ITERS", "1"))
    results = run(in_maps, n_iters=n_iters)
    _lap("stage+run")

    # host merge: candidates [T, 8 cores * 8] -> exact top-16 rescore
    cand_vals = np.empty((T, N_CORES * 8), dtype=np.float32)
    cand_idx = np.empty((T, N_CORES * 8), dtype=np.int64)
    for c in range(N_CORES):
        # val8 [128 p, tg*8+e]: token tg*128+p
        v8 = results[c]["val8"].reshape(128, NTG, 8).transpose(1, 0, 2).reshape(T, 8)
        i8 = (
            results[c]["idx8"]
            .astype(np.int64)
            .reshape(128, NTG, 8)
            .transpose(1, 0, 2)
            .reshape(T, 8)
        )
        gi = c * VSH + i8
        cand_vals[:, c * 8 : (c + 1) * 8] = v8
        cand_idx[:, c * 8 : (c + 1) * 8] = gi

    # top-16 candidates per token by (noisy) device score
    part = np.argpartition(-cand_vals, TOPC - 1, axis=1)[:, :TOPC]
    rows = np.arange(T)[:, None]
    idx16 = cand_idx[rows, part]  # [T, 16] global rows (may repeat)

    # exact fp32 rescore of the 16 candidates (chunked gather)
    s16 = np.empty((T, TOPC), dtype=np.float32)
    CH = 1024
    for t0 in range(0, T, CH):
        sl = slice(t0, t0 + CH)
        e16 = embed_table[idx16[sl]]  # [CH, 16, D]
        s16[sl] = np.einsum("tkd,td->tk", e16, x[sl], optimize=True)

    order = np.argsort(-s16, axis=1)
    best = order[:, 0]
    win_idx = idx16[rows[:, 0], best]
    # margin between best and the best *different* row (dedupe repeats)
    s_sorted = s16[rows, order]
    i_sorted = idx16[rows, order]
    diff = i_sorted != i_sorted[:, :1]
    first_diff = np.argmax(diff, axis=1)
    has_diff = diff[rows[:, 0], first_diff]
    second_val = np.where(has_diff, s_sorted[rows[:, 0], first_diff], -np.inf)
    margin = s_sorted[:, 0] - second_val

    # safety net: exact (reference-style fp32) rescore of low-margin tokens
    flagged = np.where(margin < THETA)[0]
    if flagged.size:
        import jax.numpy as jnp
        import jax as _jax

        with _jax.default_device(_jax.devices("cpu")[0]):
            s = jnp.einsum(
                "td,vd->tv",
                jnp.asarray(x[flagged]),
                jnp.asarray(embed_table),
            )
            win_idx[flagged] = np.asarray(jnp.argmax(s, axis=-1))

    _lap(f"merge+rescore ({flagged.size} flagged)")
    out = embed_table[win_idx].reshape(B, S, D)
    _lap("gather")
    return out


# revision 11
# speedup vs baseline: 1.2974x; 1.2974x over previous
"""dX-privacy embedding snap (argmax over vocab of noisy-embedding scores)
for Trainium2, 8 NeuronCores.

Distribution: vocab-sharded. Core c owns rows [c*4000, (c+1)*4000) and
scores all 8192 tokens against its shard.

Device math runs in fp8e4 (e4m3) with perf_mode=DoubleRow: each matmul
contracts 256 dims (2 fp8 rows per PE cell) over 500 vocab columns (stored
at stride 512 to satisfy DoubleRow's step%16 AP rule), all 8 PSUM banks
accumulate one 128-token tile x 4000 vocab scores over 16 k-pair steps. The
whole E^T shard (~16.8 MB fp8) stays resident in SBUF; x^T tiles stream per
128-token block. DVE max/max_index emit top-8 (value, index) per token per
core.

fp8 rounding gives score error sigma ~= 2.8 on D=4096 dot products vs a mean
top-1/top-2 gap of ~14.5, so the DEVICE top-8 is only a candidate filter:
the true winner survives into the 64 host-merged candidates unless >=8 rows
of its own shard out-noise it (P ~ 1e-10). The host then rescores the top-16
candidates per token exactly in fp32 (chunked gather + einsum) and keeps the
jnp full-vocab fp32 rescore (reference arithmetic) for tokens whose exact
margin < 0.05, which makes the final argmax match the fp32 reference.
"""

import sys, os, time

sys.path.insert(0, "/opt/trn_rl_repo")
import numpy as np

import bass_rust
import concourse.bass as bass
import concourse.mybir as mybir
from concourse import tile


f32 = mybir.dt.float32
f16 = mybir.dt.float16
f8 = mybir.dt.float8e4
u32 = mybir.dt.uint32

B, S, D, V = 4, 2048, 4096, 32000
T = B * S  # 8192 tokens
N_CORES = 8
VSH = V // N_CORES  # 4000 vocab rows per core (no padding)
KP = D // 256  # 16 k-pair steps (DoubleRow contracts 256 dims/matmul)
NVB = 8  # v blocks per core (one PSUM bank each)
NV = 500  # live v block width (stored at stride 512: DoubleRow step%16==0)
NVS = 512  # storage stride of a v block half
NTG = T // 128  # 64 token tiles of 128
THETA = 0.05  # host full-rescore margin on exact fp32 candidate scores
TOPC = 16  # candidates exactly rescored per token

_mwfix_ctr = [0]


def _legalize_multiwaits(nc, max_waits=1):
    """walrus encodes at most one sem wait per instruction; split multi-wait
    instructions by inserting single-wait NOPs before them (same engine)."""
    for fn in nc.m.functions:
        for bb in fn.blocks:
            insts = list(bb.instructions)
            out = []
            changed = False
            for inst in insts:
                si = inst.sync_info
                ow = list(si.on_wait) if si is not None and si.on_wait else []
                if len(ow) > max_waits:
                    for wentry in ow[:-max_waits]:
                        _mwfix_ctr[0] += 1
                        nop = mybir.InstNoOp(
                            name=f"mwfix-{_mwfix_ctr[0]}", ins=[], outs=[]
                        )
                        nop.engine = inst.engine
                        nop.sync_info = bass_rust.SyncInfo(
                            on_wait=[wentry], on_update=[]
                        )
                        out.append(nop)
                    si.on_wait = ow[-max_waits:]
                    changed = True
                out.append(inst)
            if changed:
                bb.instructions = out


def _build_nc():
    nc = bass.Bass()
    # xt[tg, p, kp*2*128]: xt[tg, p, kp, i, t] = x8[tg*128+t, kp*256+i*128+p]
    xt_in = nc.declare_dram_parameter("xt", [NTG, 128, KP, 2, 128], f8, isOutput=False)
    # et[p, kp, vb, i, v<500] = E8[vb*500+v, kp*256+i*128+p] for this core's
    # shard; v is stored at stride 512 (cols 500..511 are zero padding)
    et_in = nc.declare_dram_parameter("et", [128, KP, NVB, 2, NVS], f8, isOutput=False)
    out_val = nc.declare_dram_parameter("val8", [128, NTG * 8], f32, isOutput=True)
    out_idx = nc.declare_dram_parameter("idx8", [128, NTG * 8], u32, isOutput=True)

    DR = mybir.MatmulPerfMode.DoubleRow

    with tile.TileContext(nc) as tc:
        with (
            tc.tile_pool(name="et", bufs=1) as etp,
            tc.tile_pool(name="xt", bufs=3) as xtp,
            tc.tile_pool(name="bnc", bufs=2) as bncp,
            tc.tile_pool(name="o8", bufs=1) as o8p,
            tc.tile_pool(name="ps", bufs=1, space="PSUM") as ps,
        ):
            # resident E^T shard: 128 KiB per partition
            et_sb = etp.tile([128, KP, NVB, 2, NVS], f8, name="et_sb")
            # kp-granular loads so tg=0 compute starts after the first chunk;
            # alternate two HWDGE queues so the 16.8 MB doesn't serialize on
            # one queue's bandwidth during the tg=0 ramp
            for kp in range(KP):
                eng = nc.sync if kp % 2 == 0 else nc.gpsimd
                eng.dma_start(et_sb[:, kp], et_in[:, kp])

            val_acc = o8p.tile([128, NTG, 8], f32, name="val_acc")
            idx_acc = o8p.tile([128, NTG, 8], u32, name="idx_acc")

            for tg in range(NTG):
                xt_t = xtp.tile([128, KP, 2, 128], f8, tag="xt", name=f"xt_{tg}")
                nc.scalar.dma_start(xt_t[:], xt_in[tg])

                psums = []
                for vb in range(NVB):
                    pst = ps.tile([128, NV], f32, tag=f"ps{vb}", name=f"ps_{tg}_{vb}")
                    psums.append(pst)
                for kp in range(KP):
                    for vb in range(NVB):
                        mm = nc.tensor.matmul(
                            psums[vb][:],
                            xt_t[:, kp],
                            et_sb[:, kp, vb, :, :NV],
                            start=(kp == 0),
                            stop=(kp == KP - 1),
                            perf_mode=DR,
                        )
                        if vb > 0:
                            # same stationary as vb=0: skip the reload
                            mm.ldweights = False
                # bounce scores out of PSUM so the banks recycle for tg+1;
                # top-8 over the full 4096-wide row off the SBUF copy
                bsc = bncp.tile([128, NVB * NV], f32, tag="bn", name=f"bn_{tg}")
                for vb in range(NVB):
                    nc.vector.tensor_copy(
                        bsc[:, vb * NV : (vb + 1) * NV], psums[vb][:]
                    )
                nc.vector.max(out=val_acc[:, tg], in_=bsc[:])
                nc.vector.max_index(
                    out=idx_acc[:, tg], in_max=val_acc[:, tg], in_values=bsc[:]
                )
            # ACT queue: keep result writes out of the E^T load FIFO
            nc.scalar.dma_start(out_val[:], val_acc[:])
            nc.scalar.dma_start(out_idx[:], idx_acc[:])
    _legalize_multiwaits(nc)
    return nc


_RUNNER = None
LAST_TIMES = None  # per-call wall times of the timed iterations


def _get_runner():
    global _RUNNER
    if _RUNNER is not None:
        return _RUNNER
    import jax
    from jax.sharding import Mesh, PartitionSpec, NamedSharding
    from jax.experimental.shard_map import shard_map
    from concourse.bass2jax import (
        _bass_exec_p,
        install_neuronx_cc_hook,
        partition_id_tensor,
    )

    nc = _build_nc()
    install_neuronx_cc_hook()
    partition_name = nc.partition_id_tensor.name if nc.partition_id_tensor else None

    in_names, out_names, out_avals, zero_outs = [], [], [], []
    for alloc in nc.m.functions[0].allocations:
        if not isinstance(alloc, mybir.MemoryLocationSet):
            continue
        name = alloc.memorylocations[0].name
        if alloc.kind == "ExternalInput":
            if name != partition_name:
                in_names.append(name)
        elif alloc.kind == "ExternalOutput":
            shape, dt = alloc.tensor_shape, mybir.dt.np(alloc.dtype)
            out_names.append(name)
            out_avals.append(jax.core.ShapedArray(shape, dt))
            zero_outs.append(np.zeros(shape, dt))

    n_params = len(in_names)
    all_in_names = list(in_names) + list(out_names)
    if partition_name is not None:
        all_in_names.append(partition_name)

    def _body(*args):
        operands = list(args)
        if partition_name is not None:
            operands.append(partition_id_tensor())
        outs = _bass_exec_p.bind(
            *operands,
            out_avals=tuple(out_avals),
            in_names=tuple(all_in_names),
            out_names=tuple(out_names),
            lowering_input_output_aliases=(),
            sim_require_finite=True,
            sim_require_nnan=True,
            nc=nc,
        )
        return tuple(outs)

    devices = jax.devices()[:N_CORES]
    mesh = Mesh(np.asarray(devices), ("core",))
    in_specs = (PartitionSpec("core"),) * (n_params + len(out_names))
    out_specs = (PartitionSpec("core"),) * len(out_names)
    fn = jax.jit(
        shard_map(
            _body, mesh=mesh, in_specs=in_specs, out_specs=out_specs, check_rep=False
        ),
        keep_unused=True,
    )

    def run(in_maps, n_iters=1):
        global LAST_TIMES
        args = []
        for name in in_names:
            shards = [
                jax.device_put(np.ascontiguousarray(in_maps[c][name]), devices[c])
                for c in range(N_CORES)
            ]
            per_shape = shards[0].shape
            gshape = (N_CORES * per_shape[0],) + tuple(per_shape[1:])
            args.append(
                jax.make_array_from_single_device_arrays(
                    gshape, NamedSharding(mesh, PartitionSpec("core")), shards
                )
            )
        zargs = []
        for z in zero_outs:
            shards = [jax.device_put(z, d) for d in devices]
            gshape = (N_CORES * z.shape[0],) + tuple(z.shape[1:])
            zargs.append(
                jax.make_array_from_single_device_arrays(
                    gshape, NamedSharding(mesh, PartitionSpec("core")), shards
                )
            )
        out = fn(*args, *zargs)
        jax.block_until_ready(out)
        globals()["_FN"] = fn
        globals()["_ARGS"] = (args, zargs)
        times = []
        for _ in range(n_iters - 1):
            t0 = time.perf_counter()
            out = fn(*args, *zargs)
            jax.block_until_ready(out)
            times.append(time.perf_counter() - t0)
        LAST_TIMES = times
        results = []
        for c in range(N_CORES):
            m = {}
            for i, name in enumerate(out_names):
                ga = np.asarray(out[i]).reshape((N_CORES,) + out_avals[i].shape)
                m[name] = ga[c]
            results.append(m)
        return results

    _RUNNER = run
    return run


def measure_exec_ns(chains=(20, 120), tries=3):
    """Per-execution device time via long-chain slope (cancels the tunnel's
    ~40ms completion-poll quantization). Requires a prior kernel() call."""
    import jax

    fn = globals().get("_FN")
    args, zargs = globals().get("_ARGS")
    best = None
    for _ in range(tries):
        ts = []
        for n in chains:
            o = fn(*args, *zargs)
            jax.block_until_ready(o)  # sync point
            t0 = time.perf_counter()
            for _ in range(n):
                o = fn(*args, *zargs)
            jax.block_until_ready(o)
            ts.append(time.perf_counter() - t0)
        per = (ts[1] - ts[0]) / (chains[1] - chains[0])
        best = per if best is None else min(best, per)
    return best * 1e9


def kernel(inputs_embeds, embed_table, noise):
    verbose = os.environ.get("KERNEL_VERBOSE")
    _t = [time.time()]

    def _lap(msg):
        if verbose:
            t = time.time()
            print(f"[kernel] {msg}: {t - _t[0]:.1f}s", flush=True)
            _t[0] = t

    inputs_embeds = np.asarray(inputs_embeds)
    embed_table = np.asarray(embed_table)
    noise = np.asarray(noise)
    f8np = mybir.dt.np(f8)

    # host prep (layout only)
    x = (inputs_embeds + noise).reshape(T, D).astype(np.float32)
    x8 = x.astype(f8np)
    # xt[tg, p, kp, i, t] = x8[tg*128+t, kp*256+i*128+p]
    xt = np.ascontiguousarray(
        x8.reshape(NTG, 128, KP, 2, 128).transpose(0, 4, 2, 3, 1)
    )
    e8 = embed_table.astype(f8np)
    in_maps = []
    for c in range(N_CORES):
        sh = e8[c * VSH : (c + 1) * VSH]  # [4000 rows, 4096 dims] fp8
        # et[p, kp, vb, i, v<500] = sh[vb*500+v, kp*256+i*128+p]
        et = np.zeros((128, KP, NVB, 2, NVS), dtype=f8np)
        et[..., :NV] = sh.reshape(NVB, NV, KP, 2, 128).transpose(4, 2, 0, 3, 1)
        in_maps.append({"xt": xt, "et": et})
    _lap("host prep")

    run = _get_runner()
    _lap("compile/runner")
    n_iters = int(os.environ.get("KERNEL_TIME_ITERS", "1"))
    results = run(in_maps, n_iters=n_iters)
    _lap("stage+run")

    # host merge: candidates [T, 8 cores * 8] -> exact top-16 rescore
    cand_vals = np.empty((T, N_CORES * 8), dtype=np.float32)
    cand_idx = np.empty((T, N_CORES * 8), dtype=np.int64)
    for c in range(N_CORES):
        # val8 [128 p, tg*8+e]: token tg*128+p
        v8 = results[c]["val8"].reshape(128, NTG, 8).transpose(1, 0, 2).reshape(T, 8)
        i8 = (
            results[c]["idx8"]
            .astype(np.int64)
            .reshape(128, NTG, 8)
            .transpose(1, 0, 2)
            .reshape(T, 8)
        )
        gi = c * VSH + i8
        cand_vals[:, c * 8 : (c + 1) * 8] = v8
        cand_idx[:, c * 8 : (c + 1) * 8] = gi

    # top-16 candidates per token by (noisy) device score
    part = np.argpartition(-cand_vals, TOPC - 1, axis=1)[:, :TOPC]
    rows = np.arange(T)[:, None]
    idx16 = cand_idx[rows, part]  # [T, 16] global rows (may repeat)

    # exact fp32 rescore of the 16 candidates (chunked gather)
    s16 = np.empty((T, TOPC), dtype=np.float32)
    CH = 1024
    for t0 in range(0, T, CH):
        sl = slice(t0, t0 + CH)
        e16 = embed_table[idx16[sl]]  # [CH, 16, D]
        s16[sl] = np.einsum("tkd,td->tk", e16, x[sl], optimize=True)

    order = np.argsort(-s16, axis=1)
    best = order[:, 0]
    win_idx = idx16[rows[:, 0], best]
    # margin between best and the best *different* row (dedupe repeats)
    s_sorted = s16[rows, order]
    i_sorted = idx16[rows, order]
    diff = i_sorted != i_sorted[:, :1]
    first_diff = np.argmax(diff, axis=1)
    has_diff = diff[rows[:, 0], first_diff]
    second_val = np.where(has_diff, s_sorted[rows[:, 0], first_diff], -np.inf)
    margin = s_sorted[:, 0] - second_val

    # safety net: exact (reference-style fp32) rescore of low-margin tokens
    flagged = np.where(margin < THETA)[0]
    if flagged.size:
        import jax.numpy as jnp
        import jax as _jax

        with _jax.default_device(_jax.devices("cpu")[0]):
            s = jnp.einsum(
                "td,vd->tv",
                jnp.asarray(x[flagged]),
                jnp.asarray(embed_table),
            )
            win_idx[flagged] = np.asarray(jnp.argmax(s, axis=-1))

    _lap(f"merge+rescore ({flagged.size} flagged)")
    out = embed_table[win_idx].reshape(B, S, D)
    _lap("gather")
    return out
